# revision 52
# baseline (speedup 1.0000x reference)
"""Multi-head attention (B=2, S=2048, D=1024, H=16) on 8 Trainium2 NeuronCores.

Sharding: core c -> (batch b = c//4, head-group g = c%4).  Each core computes
Q/K/V projections for its 4 heads (256 features), causal attention for those
heads over the full sequence, and a partial O-projection (its 256 attn
features x full Wo.T slice).  The host sums the 4 partial outputs per batch
and folds in the biases that commute with the reduction (bo, bv @ Wo.T).

Program structure (per core): chunk-0 projections, then one loop over the 4
query blocks.  Iteration qb runs attention for qb with the projections for
chunk qb+1 and the O-projection for qb-1 interleaved as "filler" matmul
groups between attention k-tiles: TensorE uses the cycles it would otherwise
spend stalled on ScalarE's exp stream, ScalarE stays saturated through the
whole program, and TensorE never idles long enough for the HAM clock gate to
re-throttle.

Attention processes heads in pairs.  The two heads of a pair live on SBUF
partitions 0-63 / 64-127, so their QK^T matmuls (contraction K=64) lower to
PE tile_position (0,0) / (64,0): adjacent instructions on disjoint row
groups execute concurrently in the systolic array, recovering the half of
the array a K=64 matmul leaves idle.

Host-side: all large inputs are pre-packed to the SBUF partition layout so
each DMA is 128 contiguous multi-KB runs (descriptor generation on the sync
engine was a 30us serial wall with strided source patterns).

Device-side layout (per core, all matmul operands bf16, accumulation f32):
  Q^T, K^T  [feat, tok]   (feature-on-partition; per-partition bias on DVE)
  V         [tok, feat+1] (augmented with a ones column -> PV matmul also
                           accumulates the softmax denominator)
  scores^T  [k, q] tiles  -> exp on ScalarE with fused 1/sqrt(dk) scale; no
                           max-subtraction (scores are O(5) for this data,
                           exp is exact to 2 ULP and f32 can't overflow).
                           One [128,2,QB] PSUM tile holds both heads of a
                           pair so a single ACTIVATE covers both.
  masking   multiplicative bf16 tiles after exp (pattern duplicated for the
            two heads of a pair); partially-masked tiles carry a start
            column c0 so QK/exp/PV skip the dead q-range
  attnU^T + denom = V_aug^T @ P^T accumulated over k tiles in PSUM
  normalize: DVE copy of the PSUM denom row to SBUF (the PSUM accumulator
            bit layout is not IEEE f32, so the bitwise reciprocal seed must
            not read PSUM directly) + reciprocal_approx_fast + GpSimd
            partition-broadcast + DVE mul
  O-proj    attn^T tiles stationary, Wo^T slice streaming -> partial out f32
"""

import hashlib
from contextlib import ExitStack

import ml_dtypes
import numpy as np

import concourse.bass as bass
import concourse.tile as tile
from concourse import bacc, mybir
from concourse.bass_utils import run_bass_kernel_spmd

B, S, D, H = 2, 2048, 1024, 16
DK = D // H                  # 64 head dim
NCORE = 8
GROUPS = NCORE // B          # 4 head-groups per batch
HPC = H // GROUPS            # 4 heads per core
FPC = HPC * DK               # 256 features per core
FT = FPC // 128              # 2 feature tiles per core (also: head pairs)
DT = D // 128                # 8 d_in tiles
TT = S // 128                # 16 token tiles (k tiles)
QB = 512                     # query block (free-dim) size in attention
NQB = S // QB                # 4 query blocks
NCH = 512                    # psum free-dim chunk for projections
TPC = QB // 128              # token tiles per chunk
BF = mybir.dt.bfloat16
F32 = mybir.dt.float32
BFNP = ml_dtypes.bfloat16

# module-level knobs for test.py
PROFILE = False
TRACE_CORES = None
LAST_RESULT = None

_program_cache: dict = {}


def _classify_mask(mask2d: np.ndarray):
    """Classify (S, S) keep-mask into per-(qblock, ktile) modes.

    Returns (plan, patterns): plan[qb] is a list of (kt, mask_id|None, c0, c1)
    for tiles that are at least partially kept, where c0 is the first
    q-column (within the block) with any kept key and [c0, c1) the strip
    needing a multiplicative mask; patterns is a list of [128, w] bf16
    multiplicative mask tiles (k on partitions, q free).
    """
    keep = np.asarray(mask2d) != 0
    patterns = []
    pattern_ids = {}
    plan = []
    for qb in range(NQB):
        row = []
        for kt in range(TT):
            blk = keep[qb * QB:(qb + 1) * QB, kt * 128:(kt + 1) * 128].T
            if not blk.any():
                continue
            if blk.all():
                row.append((kt, None, 0, 0))
                continue
            anyk = blk.any(axis=0)
            allk = blk.all(axis=0)
            c0 = int(np.flatnonzero(anyk)[0])
            notall = np.flatnonzero(~allk)
            c1 = int(notall[-1]) + 1 if notall.size else c0
            pat = blk[:, c0:c1]
            key = pat.tobytes()
            mid = pattern_ids.get(key)
            if mid is None:
                mid = len(patterns)
                pattern_ids[key] = mid
                patterns.append(pat.astype(BFNP))
            row.append((kt, mid, c0, c1))
        plan.append(row)
    return plan, patterns


def build_program(plan, npat, pw):
    nc = bacc.Bacc("TRN2", target_bir_lowering=False, debug=False,
                   num_devices=NCORE)
    # all big inputs pre-packed host-side to [partition, linear] layout
    qT = nc.dram_tensor("qT", (NQB, 128, DT * QB), BF, kind="ExternalInput").ap()
    kT = nc.dram_tensor("kT", (NQB, 128, DT * QB), BF, kind="ExternalInput").ap()
    vT = nc.dram_tensor("vT", (NQB, 128, DT * QB), BF, kind="ExternalInput").ap()
    wqT = nc.dram_tensor("wqT", (128, DT * FPC), BF, kind="ExternalInput").ap()
    wkT = nc.dram_tensor("wkT", (128, DT * FPC), BF, kind="ExternalInput").ap()
    wvT = nc.dram_tensor("wvT", (128, DT * FPC), BF, kind="ExternalInput").ap()
    woT = nc.dram_tensor("woT", (128, FT * D), BF, kind="ExternalInput").ap()
    bqk = nc.dram_tensor("bqk", (2, FT, 128), F32, kind="ExternalInput").ap()
    masks = None
    if npat:
        masks = nc.dram_tensor("masks", (npat, 128, 2 * pw), BF,
                               kind="ExternalInput").ap()
    out = nc.dram_tensor("out", (S, D), BF, kind="ExternalOutput").ap()

    with tile.TileContext(nc) as tc, ExitStack() as ctx:
        singles = ctx.enter_context(tc.tile_pool(name="singles", bufs=1))
        ppool = ctx.enter_context(tc.tile_pool(name="ppool", bufs=6))
        npool = ctx.enter_context(tc.tile_pool(name="npool", bufs=4))
        opool = ctx.enter_context(tc.tile_pool(name="opool", bufs=4))
        psacc = ctx.enter_context(tc.tile_pool(name="psacc", bufs=2, space="PSUM"))
        psS = ctx.enter_context(tc.tile_pool(name="psS", bufs=2, space="PSUM"))
        psPV = ctx.enter_context(tc.tile_pool(name="psPV", bufs=1, space="PSUM"))

        # ---- SBUF residents ----
        wq_sb = singles.tile([128, DT, FPC], BF)
        wk_sb = singles.tile([128, DT, FPC], BF)
        wv_sb = singles.tile([128, DT, FPC], BF)
        wo_sb = singles.tile([128, FT, D], BF)
        bias_sb = singles.tile([128, 2, FT], F32)
        warm_sb = singles.tile([128, 640], BF)
        mask_sb = None
        if npat:
            mask_sb = singles.tile([128, npat, 2 * pw], BF, name="mask_sb")
        q_sb = singles.tile([128, FT, S], BF)
        k_sb = singles.tile([128, FT, S], BF)
        attn_sb = singles.tile([128, FT, S], BF)
        v_sb = singles.tile([128, TT, HPC, DK + 1], BF)
        # chunk-major input staging: [p, chunk, d_tile, tok-in-chunk]
        xv_sb = singles.tile([128, NQB, DT, QB], BF)
        xk_sb = singles.tile([128, NQB, DT, QB], BF)
        xq_sb = singles.tile([128, NQB, DT, QB], BF)

        # ---- DMA issue order: the V-projection's inputs lead (wv + first
        # half-chunk of xv), constants slot in where their first use allows,
        # later chunks stream behind.  Every transfer is 128 contiguous runs.
        # spread the critical head transfers over three DMA paths so their
        # descriptor generation and transfers overlap: xv0 halves on the
        # sync and scalar HWDGEs, wv on the gpsimd SWDGE
        nc.gpsimd.dma_start(wv_sb, wvT.rearrange("p (t f) -> p t f", t=DT))
        xv0_flat = xv_sb[:, 0].rearrange("p t f -> p (t f)")
        quart = DT * QB // 4
        for qi, eng in ((0, nc.sync), (1, nc.scalar), (2, nc.sync),
                        (3, nc.scalar)):
            eng.dma_start(
                xv0_flat[:, qi * quart:(qi + 1) * quart],
                vT[0, :, qi * quart:(qi + 1) * quart])
        nc.sync.dma_start(wk_sb, wkT.rearrange("p (t f) -> p t f", t=DT))
        xk0_flat = xk_sb[:, 0].rearrange("p t f -> p (t f)")
        nc.sync.dma_start(xk0_flat[:, 0:2 * quart], kT[0, :, 0:2 * quart])
        nc.scalar.dma_start(xk0_flat[:, 2 * quart:], kT[0, :, 2 * quart:])
        nc.gpsimd.dma_start(bias_sb, bqk.rearrange("a b p -> p a b"))
        if npat:
            nc.gpsimd.dma_start(mask_sb, masks.rearrange("m p f -> p m f"))
        nc.sync.dma_start(wq_sb, wqT.rearrange("p (t f) -> p t f", t=DT))
        nc.sync.dma_start(xq_sb[:, 0], qT[0].rearrange("p (t f) -> p t f", t=DT))
        for ch in range(1, NQB):
            nc.sync.dma_start(xv_sb[:, ch],
                              vT[ch].rearrange("p (t f) -> p t f", t=DT))
            nc.sync.dma_start(xk_sb[:, ch],
                              kT[ch].rearrange("p (t f) -> p t f", t=DT))
            nc.sync.dma_start(xq_sb[:, ch],
                              qT[ch].rearrange("p (t f) -> p t f", t=DT))
            if ch == 1:
                nc.scalar.dma_start(wo_sb,
                                    woT.rearrange("p (t f) -> p t f", t=FT))

        # PE warmup during the initial DMA window: keeps the HAM activity
        # monitor busy so real matmuls start at 2.4 GHz.
        nc.vector.memset(warm_sb, 0.0)
        for tt in range(TT):
            nc.vector.memset(v_sb[:, tt, :, DK:DK + 1], 1.0)
        for i in range(46):
            wps = psacc.tile([128, 256], F32, tag="acc", name="ps")
            nc.tensor.matmul(wps, lhsT=warm_sb[:, 0:128],
                             rhs=warm_sb[:, 128:384],
                             start=True, stop=True)

        # ---- matmul-group emitters (the TensorE work besides attention) ----
        def emit_vproj(ch, j):
            tt = ch * TPC + j
            ps = psacc.tile([128, FPC], F32, tag="acc", name="ps")
            for dt in range(DT):
                nc.tensor.matmul(ps,
                                 lhsT=xv_sb[:, ch, dt, j * 128:(j + 1) * 128],
                                 rhs=wv_sb[:, dt, :],
                                 start=(dt == 0), stop=(dt == DT - 1))
            nc.vector.tensor_copy(v_sb[:, tt, :, 0:DK],
                                  ps.rearrange("p (h d) -> p h d", h=HPC))

        def emit_kqproj(bi, x_sb, w_sb, y_sb, ft, ch):
            ps = psacc.tile([128, NCH], F32, tag="acc", name="ps")
            for dt in range(DT):
                nc.tensor.matmul(
                    ps,
                    lhsT=w_sb[:, dt, ft * 128:(ft + 1) * 128],
                    rhs=x_sb[:, ch, dt, :],
                    start=(dt == 0), stop=(dt == DT - 1))
            nc.vector.tensor_scalar_add(
                y_sb[:, ft, ch * NCH:(ch + 1) * NCH], ps,
                bias_sb[:, bi, ft:ft + 1])

        def emit_oproj(qt, nch, dmae=None):
            ps = psacc.tile([128, NCH], F32, tag="acc", name="ps")
            for hd in range(FT):
                nc.tensor.matmul(
                    ps,
                    lhsT=attn_sb[:, hd, qt * 128:(qt + 1) * 128],
                    rhs=wo_sb[:, hd, nch * NCH:(nch + 1) * NCH],
                    start=(hd == 0), stop=(hd == FT - 1))
            ob = opool.tile([128, NCH], BF)
            nc.vector.tensor_copy(ob, ps)
            # alternate output-DMA queues so descriptor generation does not
            # serialize on the sync engine
            eng = dmae if dmae is not None else (
                nc.sync if (qt + nch) % 2 == 0 else nc.gpsimd)
            eng.dma_start(
                out[qt * 128:(qt + 1) * 128, nch * NCH:(nch + 1) * NCH],
                ob)

        def proj_v_groups(ch):
            return [(lambda ch=ch, j=j: emit_vproj(ch, j)) for j in range(TPC)]

        def proj_k_groups(ch):
            return [(lambda ft=ft, ch=ch:
                     emit_kqproj(1, xk_sb, wk_sb, k_sb, ft, ch))
                    for ft in range(FT)]

        def proj_q_groups(ch):
            return [(lambda ft=ft, ch=ch:
                     emit_kqproj(0, xq_sb, wq_sb, q_sb, ft, ch))
                    for ft in range(FT)]

        def proj_groups(ch):
            return proj_v_groups(ch) + proj_k_groups(ch) + proj_q_groups(ch)

        def oproj_groups(ch):
            return [(lambda qt=qt, nch=nch: emit_oproj(qt, nch))
                    for qt in range(ch * TPC, (ch + 1) * TPC)
                    for nch in range(D // NCH)]

        def interleave(a, b):
            out_ = []
            for i in range(max(len(a), len(b))):
                if i < len(a):
                    out_.append(a[i])
                if i < len(b):
                    out_.append(b[i])
            return out_

        # chunk-0 projections run straight; later chunks and the
        # O-projections interleave into the attention loop as fillers
        for g in proj_groups(0):
            g()

        # filler schedule, balanced against each block's ScalarE exp load:
        # qb3's k-tiles 12-15 are the only users of chunk-3 K, so the K
        # projection for chunk 3 shifts into qb3 (which is otherwise
        # exp-bound); chunk-3's V and Q projections must finish in qb2
        # (qb3's PV and first scores need them).
        # within each list, projection groups (input-gated, ready early)
        # interleave ahead of O-projection groups (gated on the previous
        # block's normalize, which lands late) so the first filler slots of
        # a block never stall
        op1 = oproj_groups(1)
        qb_fillers = [
            proj_groups(1),
            interleave(proj_groups(2), oproj_groups(0)),
            interleave(proj_v_groups(3) + proj_q_groups(3), op1[:4]),
            interleave(proj_k_groups(3) + op1[4:], oproj_groups(2)),
        ]
        inv_sqrt_dk = float(1.0 / np.sqrt(DK))
        for qb in range(NQB):
            fillers = qb_fillers[qb]
            kts = plan[qb]
            # budget fillers per head pair so the second pair's exp-latency
            # stalls still have TensorE work available
            nslots = max(len(kts), 1)
            state = {"fi": 0, "si": 0, "budget": (len(fillers) + 1) // 2}

            def filler_slot():
                state["si"] += 1
                want = (state["si"] * state["budget"] + nslots - 1) // nslots
                lim = min(state["fi0"] + want, len(fillers))
                while state["fi"] < lim:
                    fillers[state["fi"]]()
                    state["fi"] += 1

            def start_pair(last):
                state["si"] = 0
                state["fi0"] = state["fi"]
                if last:
                    state["budget"] = len(fillers) - state["fi"]

            state["fi0"] = 0

            # ---- attention for query block qb, heads in pairs ----
            for fp in range(FT):
                if not kts:
                    continue
                start_pair(fp == FT - 1)
                h0, h1 = 2 * fp, 2 * fp + 1
                pv0 = psPV.tile([DK + 1, QB], F32)
                pv1 = psPV.tile([DK + 1, QB], F32)

                def emit_pv(n, kt, c0):
                    nc.tensor.matmul(pv0[:, c0:], lhsT=v_sb[:, kt, h0, :],
                                     rhs=pts[n][:, 0, c0:],
                                     start=(n == 0), stop=(n == len(kts) - 1))
                    nc.tensor.matmul(pv1[:, c0:], lhsT=v_sb[:, kt, h1, :],
                                     rhs=pts[n][:, 1, c0:],
                                     start=(n == 0), stop=(n == len(kts) - 1))

                pts = []
                for n, (kt, mid, c0, c1) in enumerate(kts):
                    s_ps = psS.tile([128, 2, QB], F32)
                    # QK^T for both heads: disjoint PE row groups (0,0) and
                    # (64,0) -> the two matmuls execute concurrently.
                    nc.tensor.matmul(
                        s_ps[:, 0, c0:],
                        lhsT=k_sb[0:DK, fp, kt * 128:(kt + 1) * 128],
                        rhs=q_sb[0:DK, fp, qb * QB + c0:(qb + 1) * QB],
                        start=True, stop=True)
                    nc.tensor.matmul(
                        s_ps[:, 1, c0:],
                        lhsT=k_sb[DK:128, fp, kt * 128:(kt + 1) * 128],
                        rhs=q_sb[DK:128, fp, qb * QB + c0:(qb + 1) * QB],
                        start=True, stop=True)
                    pT = ppool.tile([128, 2, QB], BF, tag="pt")
                    nc.scalar.activation(pT[:, :, c0:], s_ps[:, :, c0:],
                                         mybir.ActivationFunctionType.Exp,
                                         scale=inv_sqrt_dk)
                    if mid is not None and c1 > c0:
                        assert mask_sb is not None
                        w = c1 - c0
                        sl = pT[:, 0, c0:c1]
                        both = bass.AP(
                            tensor=sl.tensor, offset=sl.offset,
                            ap=[sl.ap[0], [QB, 2], sl.ap[-1]])
                        nc.vector.tensor_mul(
                            both, both,
                            mask_sb[:, mid, 0:2 * w].rearrange(
                                "p (j w) -> p j w", j=2))
                    pts.append(pT)
                    # fillers absorb the TensorE wait for exp(n-1) ...
                    filler_slot()
                    # ... then PV for the previous k-tile
                    if n >= 1:
                        emit_pv(n - 1, kts[n - 1][0], kts[n - 1][2])
                emit_pv(len(kts) - 1, kts[-1][0], kts[-1][2])

                # normalize: attn^T[d, q] = attnU^T[d, q] / denom[q]
                for h, pv in ((h0, pv0), (h1, pv1)):
                    pr = (h % 2) * 64
                    ft = h // 2
                    den = npool.tile([1, QB], F32, tag="den")
                    nc.vector.tensor_copy(den, pv[DK:DK + 1, :])
                    rec = npool.tile([1, QB], F32, tag="rec")
                    nc.vector.reciprocal_approx_fast(rec, den)
                    bc = npool.tile([64, QB], F32, tag="bc")
                    nc.gpsimd.partition_broadcast(bc, rec)
                    dst = attn_sb[pr:pr + DK, ft, qb * QB:(qb + 1) * QB]
                    nc.vector.tensor_mul(dst, pv[0:DK, :], bc)

            # drain any fillers not emitted in the slots
            while state["fi"] < len(fillers):
                fillers[state["fi"]]()
                state["fi"] += 1

        # final block's O-projection: spread output DMAs across all three
        # DGE queues (ScalarE is idle by now) to shorten the drain tail
        tail_engines = [nc.sync, nc.scalar, nc.gpsimd]
        i = 0
        for qt in range((NQB - 1) * TPC, NQB * TPC):
            for nch in range(D // NCH):
                emit_oproj(qt, nch, dmae=tail_engines[i % 3])
                i += 1

    nc.compile()
    return nc


def _get_program(mask2d: np.ndarray):
    key = hashlib.sha1(np.ascontiguousarray(mask2d).tobytes()).hexdigest()
    hit = _program_cache.get(key)
    if hit is not None:
        return hit
    plan, patterns = _classify_mask(mask2d)
    # duplicate each pattern so one strided DVE multiply covers both heads
    # of a pair: [128, w] -> [128, 2w] = [pat | pat]
    pw = max((p.shape[1] for p in patterns), default=0)
    nc = build_program(plan, len(patterns), pw)
    if patterns:
        pat = np.zeros((len(patterns), 128, 2 * pw), BFNP)
        for i, p in enumerate(patterns):
            w = p.shape[1]
            pat[i, :, :w] = p
            pat[i, :, w:2 * w] = p
    else:
        pat = None
    _program_cache[key] = (nc, pat)
    return nc, pat


def _pack_x(xb: np.ndarray) -> np.ndarray:
    """(S, D) activation -> (NQB, 128, DT*QB) bf16 in SBUF partition layout."""
    # x^T is (D, S) = (dt*128, nqb*QB); want [ch][p][dt, tok]
    xt = xb.T.reshape(DT, 128, NQB, QB)
    return np.ascontiguousarray(xt.transpose(2, 1, 0, 3).reshape(
        NQB, 128, DT * QB)).astype(BFNP)


def _pack_w(wT: np.ndarray) -> np.ndarray:
    """(D, FPC) weight -> (128, DT*FPC) bf16 in SBUF partition layout."""
    return np.ascontiguousarray(
        wT.reshape(DT, 128, FPC).transpose(1, 0, 2).reshape(
            128, DT * FPC)).astype(BFNP)


def _pack_wo(woT: np.ndarray) -> np.ndarray:
    """(FPC, D) weight -> (128, FT*D) bf16 in SBUF partition layout."""
    return np.ascontiguousarray(
        woT.reshape(FT, 128, D).transpose(1, 0, 2).reshape(
            128, FT * D)).astype(BFNP)


def kernel(**inputs) -> np.ndarray:
    global LAST_RESULT
    query = np.asarray(inputs["query"], np.float32)
    key = np.asarray(inputs["key"], np.float32)
    value = np.asarray(inputs["value"], np.float32)
    mask = np.asarray(inputs["mask"])
    Wq = np.asarray(inputs["Wq"], np.float32)
    bq = np.asarray(inputs["bq"], np.float32)
    Wk = np.asarray(inputs["Wk"], np.float32)
    bk = np.asarray(inputs["bk"], np.float32)
    Wv = np.asarray(inputs["Wv"], np.float32)
    bv = np.asarray(inputs["bv"], np.float32)
    Wo = np.asarray(inputs["Wo"], np.float32)
    bo = np.asarray(inputs["bo"], np.float32)

    nc, pat = _get_program(mask.reshape(S, S))

    WqT, WkT, WvT, WoT = Wq.T, Wk.T, Wv.T, Wo.T
    xP = {
        t: [_pack_x(x[b]) for b in range(B)]
        for t, x in (("qT", query), ("kT", key), ("vT", value))
    }
    in_maps = []
    for c in range(NCORE):
        b, g = divmod(c, GROUPS)
        f0 = g * FPC
        m = {
            "qT": xP["qT"][b],
            "kT": xP["kT"][b],
            "vT": xP["vT"][b],
            "wqT": _pack_w(WqT[:, f0:f0 + FPC]),
            "wkT": _pack_w(WkT[:, f0:f0 + FPC]),
            "wvT": _pack_w(WvT[:, f0:f0 + FPC]),
            "woT": _pack_wo(WoT[f0:f0 + FPC, :]),
            "bqk": np.stack([bq[f0:f0 + FPC].reshape(FT, 128),
                             bk[f0:f0 + FPC].reshape(FT, 128)]).astype(np.float32),
        }
        if pat is not None:
            m["masks"] = pat
        in_maps.append(m)

    res = run_bass_kernel_spmd(
        nc, in_maps, core_ids=list(range(NCORE)),
        trace=PROFILE,
        trace_cores=(TRACE_CORES if TRACE_CORES is not None
                     else (list(range(NCORE)) if PROFILE else None)),
    )
    LAST_RESULT = res

    host_bias = bo + bv @ WoT  # (D,) folded V/O biases, added once per batch
    out = np.empty((B, S, D), np.float32)
    for b in range(B):
        acc = res.results[b * GROUPS]["out"].astype(np.float32)
        for g in range(1, GROUPS):
            acc = acc + res.results[b * GROUPS + g]["out"].astype(np.float32)
        out[b] = acc + host_bias
    return out


# revision 53
# speedup vs baseline: 1.1682x; 1.1682x over previous
"""Multi-head attention (B=2, S=2048, D=1024, H=16) on 8 Trainium2 NeuronCores.

Sharding: core c -> (batch b = c//4, head-group g = c%4).  Each core computes
Q/K/V projections for its 4 heads (256 features), causal attention for those
heads over the full sequence, and a partial O-projection (its 256 attn
features x full Wo.T slice).  The host sums the 4 partial outputs per batch
and folds in the biases that commute with the reduction (bo, bv @ Wo.T).

Program structure (per core): chunk-0 projections, then one loop over the 4
query blocks.  Iteration qb runs attention for qb with the projections for
chunk qb+1 and the O-projection for qb-1 interleaved as "filler" matmul
groups between attention k-tiles: TensorE uses the cycles it would otherwise
spend stalled on ScalarE's exp stream, ScalarE stays saturated through the
whole program, and TensorE never idles long enough for the HAM clock gate to
re-throttle.

Attention processes heads in pairs.  The two heads of a pair live on SBUF
partitions 0-63 / 64-127, so their QK^T matmuls (contraction K=64) lower to
PE tile_position (0,0) / (64,0): adjacent instructions on disjoint row
groups execute concurrently in the systolic array, recovering the half of
the array a K=64 matmul leaves idle.

Host-side: all large inputs are pre-packed to the SBUF partition layout so
each DMA is 128 contiguous multi-KB runs (descriptor generation on the sync
engine was a 30us serial wall with strided source patterns).

Device-side layout (per core, all matmul operands bf16, accumulation f32):
  Q^T, K^T  [feat, tok]   (feature-on-partition; per-partition bias on DVE)
  V         [tok, feat+1] (augmented with a ones column -> PV matmul also
                           accumulates the softmax denominator)
  scores^T  [k, q] tiles  -> exp on ScalarE with fused 1/sqrt(dk) scale; no
                           max-subtraction (scores are O(5) for this data,
                           exp is exact to 2 ULP and f32 can't overflow).
                           One [128,2,QB] PSUM tile holds both heads of a
                           pair so a single ACTIVATE covers both.
  masking   multiplicative bf16 tiles after exp (pattern duplicated for the
            two heads of a pair); partially-masked tiles carry a start
            column c0 so QK/exp/PV skip the dead q-range
  attnU^T + denom = V_aug^T @ P^T accumulated over k tiles in PSUM
  normalize: DVE copy of the PSUM denom row to SBUF (the PSUM accumulator
            bit layout is not IEEE f32, so the bitwise reciprocal seed must
            not read PSUM directly) + reciprocal_approx_fast + GpSimd
            partition-broadcast + DVE mul
  O-proj    attn^T tiles stationary, Wo^T slice streaming -> partial out f32
"""

import hashlib
from contextlib import ExitStack

import ml_dtypes
import numpy as np

import concourse.bass as bass
import concourse.tile as tile
from concourse import bacc, mybir
from concourse.bass_utils import run_bass_kernel_spmd

B, S, D, H = 2, 2048, 1024, 16
DK = D // H                  # 64 head dim
NCORE = 8
GROUPS = NCORE // B          # 4 head-groups per batch
HPC = H // GROUPS            # 4 heads per core
FPC = HPC * DK               # 256 features per core
FT = FPC // 128              # 2 feature tiles per core (also: head pairs)
DT = D // 128                # 8 d_in tiles
TT = S // 128                # 16 token tiles (k tiles)
QB = 512                     # query block (free-dim) size in attention
NQB = S // QB                # 4 query blocks
NCH = 512                    # psum free-dim chunk for projections
TPC = QB // 128              # token tiles per chunk
BF = mybir.dt.bfloat16
F32 = mybir.dt.float32
BFNP = ml_dtypes.bfloat16

# module-level knobs for test.py
PROFILE = False
TRACE_CORES = None
LAST_RESULT = None

_program_cache: dict = {}


def _classify_mask(mask2d: np.ndarray):
    """Classify (S, S) keep-mask into per-(qblock, ktile) modes.

    Returns (plan, patterns): plan[qb] is a list of (kt, mask_id|None, c0, c1)
    for tiles that are at least partially kept, where c0 is the first
    q-column (within the block) with any kept key and [c0, c1) the strip
    needing a multiplicative mask; patterns is a list of [128, w] bf16
    multiplicative mask tiles (k on partitions, q free).
    """
    keep = np.asarray(mask2d) != 0
    patterns = []
    pattern_ids = {}
    plan = []
    for qb in range(NQB):
        row = []
        for kt in range(TT):
            blk = keep[qb * QB:(qb + 1) * QB, kt * 128:(kt + 1) * 128].T
            if not blk.any():
                continue
            if blk.all():
                row.append((kt, None, 0, 0))
                continue
            anyk = blk.any(axis=0)
            allk = blk.all(axis=0)
            c0 = int(np.flatnonzero(anyk)[0])
            notall = np.flatnonzero(~allk)
            c1 = int(notall[-1]) + 1 if notall.size else c0
            pat = blk[:, c0:c1]
            key = pat.tobytes()
            mid = pattern_ids.get(key)
            if mid is None:
                mid = len(patterns)
                pattern_ids[key] = mid
                patterns.append(pat.astype(BFNP))
            row.append((kt, mid, c0, c1))
        plan.append(row)
    return plan, patterns


def build_program(plan, npat, pw):
    nc = bacc.Bacc("TRN2", target_bir_lowering=False, debug=False,
                   num_devices=NCORE)
    # all big inputs pre-packed host-side to [partition, linear] layout
    qT = nc.dram_tensor("qT", (NQB, 128, DT * QB), BF, kind="ExternalInput").ap()
    kT = nc.dram_tensor("kT", (NQB, 128, DT * QB), BF, kind="ExternalInput").ap()
    vT = nc.dram_tensor("vT", (NQB, 128, DT * QB), BF, kind="ExternalInput").ap()
    wqT = nc.dram_tensor("wqT", (128, DT * FPC), BF, kind="ExternalInput").ap()
    wkT = nc.dram_tensor("wkT", (128, DT * FPC), BF, kind="ExternalInput").ap()
    wvT = nc.dram_tensor("wvT", (128, DT * FPC), BF, kind="ExternalInput").ap()
    woT = nc.dram_tensor("woT", (128, FT * D), BF, kind="ExternalInput").ap()
    bqk = nc.dram_tensor("bqk", (2, FT, 128), F32, kind="ExternalInput").ap()
    masks = None
    if npat:
        masks = nc.dram_tensor("masks", (npat, 128, 2 * pw), BF,
                               kind="ExternalInput").ap()
    out = nc.dram_tensor("out", (S, D), BF, kind="ExternalOutput").ap()

    with tile.TileContext(nc) as tc, ExitStack() as ctx:
        singles = ctx.enter_context(tc.tile_pool(name="singles", bufs=1))
        ppool = ctx.enter_context(tc.tile_pool(name="ppool", bufs=6))
        npool = ctx.enter_context(tc.tile_pool(name="npool", bufs=4))
        opool = ctx.enter_context(tc.tile_pool(name="opool", bufs=4))
        psacc = ctx.enter_context(tc.tile_pool(name="psacc", bufs=2, space="PSUM"))
        psS = ctx.enter_context(tc.tile_pool(name="psS", bufs=2, space="PSUM"))
        psPV = ctx.enter_context(tc.tile_pool(name="psPV", bufs=1, space="PSUM"))

        # ---- SBUF residents ----
        wq_sb = singles.tile([128, DT, FPC], BF)
        wk_sb = singles.tile([128, DT, FPC], BF)
        wv_sb = singles.tile([128, DT, FPC], BF)
        wo_sb = singles.tile([128, FT, D], BF)
        bias_sb = singles.tile([128, 2, FT], F32)
        warm_sb = singles.tile([128, 640], BF)
        mask_sb = None
        if npat:
            mask_sb = singles.tile([128, npat, 2 * pw], BF, name="mask_sb")
        q_sb = singles.tile([128, FT, S], BF)
        k_sb = singles.tile([128, FT, S], BF)
        attn_sb = singles.tile([128, FT, S], BF)
        v_sb = singles.tile([128, TT, HPC, DK + 1], BF)
        # chunk-major input staging: [p, chunk, d_tile, tok-in-chunk]
        xv_sb = singles.tile([128, NQB, DT, QB], BF)
        xk_sb = singles.tile([128, NQB, DT, QB], BF)
        xq_sb = singles.tile([128, NQB, DT, QB], BF)

        # ---- DMA issue order: the V-projection's inputs lead (wv + first
        # half-chunk of xv), constants slot in where their first use allows,
        # later chunks stream behind.  Every transfer is 128 contiguous runs.
        # spread the critical head transfers over three DMA paths so their
        # descriptor generation and transfers overlap: xv0 halves on the
        # sync and scalar HWDGEs, wv on the gpsimd SWDGE
        nc.gpsimd.dma_start(wv_sb, wvT.rearrange("p (t f) -> p t f", t=DT))
        xv0_flat = xv_sb[:, 0].rearrange("p t f -> p (t f)")
        quart = DT * QB // 4
        for qi, eng in ((0, nc.sync), (1, nc.scalar), (2, nc.sync),
                        (3, nc.scalar)):
            eng.dma_start(
                xv0_flat[:, qi * quart:(qi + 1) * quart],
                vT[0, :, qi * quart:(qi + 1) * quart])
        nc.sync.dma_start(wk_sb, wkT.rearrange("p (t f) -> p t f", t=DT))
        nc.sync.dma_start(xk_sb[:, 0], kT[0].rearrange("p (t f) -> p t f", t=DT))
        nc.sync.dma_start(bias_sb, bqk.rearrange("a b p -> p a b"))
        if npat:
            nc.sync.dma_start(mask_sb, masks.rearrange("m p f -> p m f"))
        nc.sync.dma_start(wq_sb, wqT.rearrange("p (t f) -> p t f", t=DT))
        nc.sync.dma_start(xq_sb[:, 0], qT[0].rearrange("p (t f) -> p t f", t=DT))
        nc.sync.dma_start(wo_sb, woT.rearrange("p (t f) -> p t f", t=FT))
        for ch in range(1, NQB):
            nc.sync.dma_start(xv_sb[:, ch],
                              vT[ch].rearrange("p (t f) -> p t f", t=DT))
            nc.sync.dma_start(xk_sb[:, ch],
                              kT[ch].rearrange("p (t f) -> p t f", t=DT))
            nc.sync.dma_start(xq_sb[:, ch],
                              qT[ch].rearrange("p (t f) -> p t f", t=DT))

        # PE warmup during the initial DMA window: keeps the HAM activity
        # monitor busy so real matmuls start at 2.4 GHz.
        nc.vector.memset(warm_sb, 0.0)
        for tt in range(TT):
            nc.vector.memset(v_sb[:, tt, :, DK:DK + 1], 1.0)
        for i in range(46):
            wps = psacc.tile([128, 256], F32, tag="acc", name="ps")
            nc.tensor.matmul(wps, lhsT=warm_sb[:, 0:128],
                             rhs=warm_sb[:, 128:384],
                             start=True, stop=True)

        # ---- matmul-group emitters (the TensorE work besides attention) ----
        def emit_vproj(ch, j):
            tt = ch * TPC + j
            ps = psacc.tile([128, FPC], F32, tag="acc", name="ps")
            for dt in range(DT):
                nc.tensor.matmul(ps,
                                 lhsT=xv_sb[:, ch, dt, j * 128:(j + 1) * 128],
                                 rhs=wv_sb[:, dt, :],
                                 start=(dt == 0), stop=(dt == DT - 1))
            nc.vector.tensor_copy(v_sb[:, tt, :, 0:DK],
                                  ps.rearrange("p (h d) -> p h d", h=HPC))

        def emit_kqproj(bi, x_sb, w_sb, y_sb, ft, ch):
            ps = psacc.tile([128, NCH], F32, tag="acc", name="ps")
            for dt in range(DT):
                nc.tensor.matmul(
                    ps,
                    lhsT=w_sb[:, dt, ft * 128:(ft + 1) * 128],
                    rhs=x_sb[:, ch, dt, :],
                    start=(dt == 0), stop=(dt == DT - 1))
            nc.vector.tensor_scalar_add(
                y_sb[:, ft, ch * NCH:(ch + 1) * NCH], ps,
                bias_sb[:, bi, ft:ft + 1])

        def emit_oproj(qt, nch, dmae=None):
            ps = psacc.tile([128, NCH], F32, tag="acc", name="ps")
            for hd in range(FT):
                nc.tensor.matmul(
                    ps,
                    lhsT=attn_sb[:, hd, qt * 128:(qt + 1) * 128],
                    rhs=wo_sb[:, hd, nch * NCH:(nch + 1) * NCH],
                    start=(hd == 0), stop=(hd == FT - 1))
            ob = opool.tile([128, NCH], BF)
            nc.vector.tensor_copy(ob, ps)
            # alternate output-DMA queues so descriptor generation does not
            # serialize on the sync engine
            eng = dmae if dmae is not None else (
                nc.sync if (qt + nch) % 2 == 0 else nc.gpsimd)
            eng.dma_start(
                out[qt * 128:(qt + 1) * 128, nch * NCH:(nch + 1) * NCH],
                ob)

        def proj_v_groups(ch):
            return [(lambda ch=ch, j=j: emit_vproj(ch, j)) for j in range(TPC)]

        def proj_k_groups(ch):
            return [(lambda ft=ft, ch=ch:
                     emit_kqproj(1, xk_sb, wk_sb, k_sb, ft, ch))
                    for ft in range(FT)]

        def proj_q_groups(ch):
            return [(lambda ft=ft, ch=ch:
                     emit_kqproj(0, xq_sb, wq_sb, q_sb, ft, ch))
                    for ft in range(FT)]

        def proj_groups(ch):
            return proj_v_groups(ch) + proj_k_groups(ch) + proj_q_groups(ch)

        def oproj_groups(ch):
            return [(lambda qt=qt, nch=nch: emit_oproj(qt, nch))
                    for qt in range(ch * TPC, (ch + 1) * TPC)
                    for nch in range(D // NCH)]

        def interleave(a, b):
            out_ = []
            for i in range(max(len(a), len(b))):
                if i < len(a):
                    out_.append(a[i])
                if i < len(b):
                    out_.append(b[i])
            return out_

        # chunk-0 projections run straight; later chunks and the
        # O-projections interleave into the attention loop as fillers
        for g in proj_groups(0):
            g()

        # filler schedule, balanced against each block's ScalarE exp load:
        # qb3's k-tiles 12-15 are the only users of chunk-3 K, so the K
        # projection for chunk 3 shifts into qb3 (which is otherwise
        # exp-bound); chunk-3's V and Q projections must finish in qb2
        # (qb3's PV and first scores need them).
        # within each list, projection groups (input-gated, ready early)
        # interleave ahead of O-projection groups (gated on the previous
        # block's normalize, which lands late) so the first filler slots of
        # a block never stall
        op1 = oproj_groups(1)
        qb_fillers = [
            proj_groups(1),
            interleave(proj_groups(2), oproj_groups(0)),
            interleave(proj_v_groups(3) + proj_q_groups(3), op1[:4]),
            interleave(proj_k_groups(3) + op1[4:], oproj_groups(2)),
        ]
        inv_sqrt_dk = float(1.0 / np.sqrt(DK))
        for qb in range(NQB):
            fillers = qb_fillers[qb]
            kts = plan[qb]
            # budget fillers per head pair so the second pair's exp-latency
            # stalls still have TensorE work available
            nslots = max(len(kts), 1)
            state = {"fi": 0, "si": 0, "budget": (len(fillers) + 1) // 2}

            def filler_slot():
                state["si"] += 1
                want = (state["si"] * state["budget"] + nslots - 1) // nslots
                lim = min(state["fi0"] + want, len(fillers))
                while state["fi"] < lim:
                    fillers[state["fi"]]()
                    state["fi"] += 1

            def start_pair(last):
                state["si"] = 0
                state["fi0"] = state["fi"]
                if last:
                    state["budget"] = len(fillers) - state["fi"]

            state["fi0"] = 0

            # ---- attention for query block qb, heads in pairs ----
            for fp in range(FT):
                if not kts:
                    continue
                start_pair(fp == FT - 1)
                h0, h1 = 2 * fp, 2 * fp + 1
                pv0 = psPV.tile([DK + 1, QB], F32)
                pv1 = psPV.tile([DK + 1, QB], F32)

                def emit_pv(n, kt, c0):
                    nc.tensor.matmul(pv0[:, c0:], lhsT=v_sb[:, kt, h0, :],
                                     rhs=pts[n][:, 0, c0:],
                                     start=(n == 0), stop=(n == len(kts) - 1))
                    nc.tensor.matmul(pv1[:, c0:], lhsT=v_sb[:, kt, h1, :],
                                     rhs=pts[n][:, 1, c0:],
                                     start=(n == 0), stop=(n == len(kts) - 1))

                pts = []
                for n, (kt, mid, c0, c1) in enumerate(kts):
                    s_ps = psS.tile([128, 2, QB], F32)
                    # QK^T for both heads: disjoint PE row groups (0,0) and
                    # (64,0) -> the two matmuls execute concurrently.
                    nc.tensor.matmul(
                        s_ps[:, 0, c0:],
                        lhsT=k_sb[0:DK, fp, kt * 128:(kt + 1) * 128],
                        rhs=q_sb[0:DK, fp, qb * QB + c0:(qb + 1) * QB],
                        start=True, stop=True)
                    nc.tensor.matmul(
                        s_ps[:, 1, c0:],
                        lhsT=k_sb[DK:128, fp, kt * 128:(kt + 1) * 128],
                        rhs=q_sb[DK:128, fp, qb * QB + c0:(qb + 1) * QB],
                        start=True, stop=True)
                    pT = ppool.tile([128, 2, QB], BF, tag="pt")
                    nc.scalar.activation(pT[:, :, c0:], s_ps[:, :, c0:],
                                         mybir.ActivationFunctionType.Exp,
                                         scale=inv_sqrt_dk)
                    if mid is not None and c1 > c0:
                        assert mask_sb is not None
                        w = c1 - c0
                        sl = pT[:, 0, c0:c1]
                        both = bass.AP(
                            tensor=sl.tensor, offset=sl.offset,
                            ap=[sl.ap[0], [QB, 2], sl.ap[-1]])
                        nc.vector.tensor_mul(
                            both, both,
                            mask_sb[:, mid, 0:2 * w].rearrange(
                                "p (j w) -> p j w", j=2))
                    pts.append(pT)
                    # fillers absorb the TensorE wait for exp(n-1) ...
                    filler_slot()
                    # ... then PV for the previous k-tile
                    if n >= 1:
                        emit_pv(n - 1, kts[n - 1][0], kts[n - 1][2])
                emit_pv(len(kts) - 1, kts[-1][0], kts[-1][2])

                # normalize: attn^T[d, q] = attnU^T[d, q] / denom[q]
                for h, pv in ((h0, pv0), (h1, pv1)):
                    pr = (h % 2) * 64
                    ft = h // 2
                    den = npool.tile([1, QB], F32, tag="den")
                    nc.vector.tensor_copy(den, pv[DK:DK + 1, :])
                    rec = npool.tile([1, QB], F32, tag="rec")
                    nc.vector.reciprocal_approx_fast(rec, den)
                    bc = npool.tile([64, QB], F32, tag="bc")
                    nc.gpsimd.partition_broadcast(bc, rec)
                    dst = attn_sb[pr:pr + DK, ft, qb * QB:(qb + 1) * QB]
                    nc.vector.tensor_mul(dst, pv[0:DK, :], bc)

            # drain any fillers not emitted in the slots
            while state["fi"] < len(fillers):
                fillers[state["fi"]]()
                state["fi"] += 1

        # final block's O-projection: spread output DMAs across all three
        # DGE queues (ScalarE is idle by now) to shorten the drain tail
        tail_engines = [nc.sync, nc.scalar, nc.gpsimd]
        i = 0
        for qt in range((NQB - 1) * TPC, NQB * TPC):
            for nch in range(D // NCH):
                emit_oproj(qt, nch, dmae=tail_engines[i % 3])
                i += 1

    nc.compile()
    return nc


def _get_program(mask2d: np.ndarray):
    key = hashlib.sha1(np.ascontiguousarray(mask2d).tobytes()).hexdigest()
    hit = _program_cache.get(key)
    if hit is not None:
        return hit
    plan, patterns = _classify_mask(mask2d)
    # duplicate each pattern so one strided DVE multiply covers both heads
    # of a pair: [128, w] -> [128, 2w] = [pat | pat]
    pw = max((p.shape[1] for p in patterns), default=0)
    nc = build_program(plan, len(patterns), pw)
    if patterns:
        pat = np.zeros((len(patterns), 128, 2 * pw), BFNP)
        for i, p in enumerate(patterns):
            w = p.shape[1]
            pat[i, :, :w] = p
            pat[i, :, w:2 * w] = p
    else:
        pat = None
    _program_cache[key] = (nc, pat)
    return nc, pat


def _pack_x(xb: np.ndarray) -> np.ndarray:
    """(S, D) activation -> (NQB, 128, DT*QB) bf16 in SBUF partition layout."""
    # x^T is (D, S) = (dt*128, nqb*QB); want [ch][p][dt, tok]
    xt = xb.T.reshape(DT, 128, NQB, QB)
    return np.ascontiguousarray(xt.transpose(2, 1, 0, 3).reshape(
        NQB, 128, DT * QB)).astype(BFNP)


def _pack_w(wT: np.ndarray) -> np.ndarray:
    """(D, FPC) weight -> (128, DT*FPC) bf16 in SBUF partition layout."""
    return np.ascontiguousarray(
        wT.reshape(DT, 128, FPC).transpose(1, 0, 2).reshape(
            128, DT * FPC)).astype(BFNP)


def _pack_wo(woT: np.ndarray) -> np.ndarray:
    """(FPC, D) weight -> (128, FT*D) bf16 in SBUF partition layout."""
    return np.ascontiguousarray(
        woT.reshape(FT, 128, D).transpose(1, 0, 2).reshape(
            128, FT * D)).astype(BFNP)


def kernel(**inputs) -> np.ndarray:
    global LAST_RESULT
    query = np.asarray(inputs["query"], np.float32)
    key = np.asarray(inputs["key"], np.float32)
    value = np.asarray(inputs["value"], np.float32)
    mask = np.asarray(inputs["mask"])
    Wq = np.asarray(inputs["Wq"], np.float32)
    bq = np.asarray(inputs["bq"], np.float32)
    Wk = np.asarray(inputs["Wk"], np.float32)
    bk = np.asarray(inputs["bk"], np.float32)
    Wv = np.asarray(inputs["Wv"], np.float32)
    bv = np.asarray(inputs["bv"], np.float32)
    Wo = np.asarray(inputs["Wo"], np.float32)
    bo = np.asarray(inputs["bo"], np.float32)

    nc, pat = _get_program(mask.reshape(S, S))

    WqT, WkT, WvT, WoT = Wq.T, Wk.T, Wv.T, Wo.T
    xP = {
        t: [_pack_x(x[b]) for b in range(B)]
        for t, x in (("qT", query), ("kT", key), ("vT", value))
    }
    in_maps = []
    for c in range(NCORE):
        b, g = divmod(c, GROUPS)
        f0 = g * FPC
        m = {
            "qT": xP["qT"][b],
            "kT": xP["kT"][b],
            "vT": xP["vT"][b],
            "wqT": _pack_w(WqT[:, f0:f0 + FPC]),
            "wkT": _pack_w(WkT[:, f0:f0 + FPC]),
            "wvT": _pack_w(WvT[:, f0:f0 + FPC]),
            "woT": _pack_wo(WoT[f0:f0 + FPC, :]),
            "bqk": np.stack([bq[f0:f0 + FPC].reshape(FT, 128),
                             bk[f0:f0 + FPC].reshape(FT, 128)]).astype(np.float32),
        }
        if pat is not None:
            m["masks"] = pat
        in_maps.append(m)

    res = run_bass_kernel_spmd(
        nc, in_maps, core_ids=list(range(NCORE)),
        trace=PROFILE,
        trace_cores=(TRACE_CORES if TRACE_CORES is not None
                     else (list(range(NCORE)) if PROFILE else None)),
    )
    LAST_RESULT = res

    host_bias = bo + bv @ WoT  # (D,) folded V/O biases, added once per batch
    out = np.empty((B, S, D), np.float32)
    for b in range(B):
        acc = res.results[b * GROUPS]["out"].astype(np.float32)
        for g in range(1, GROUPS):
            acc = acc + res.results[b * GROUPS + g]["out"].astype(np.float32)
        out[b] = acc + host_bias
    return out
